# revision 1
# baseline (speedup 1.0000x reference)
"""NT-Xent loss kernel for 8 Trainium2 NeuronCores (Bass/Tile).

Strategy (data-parallel rows, SPMD):
  - Host: concat z_i,z_j -> reps [8192, 512], cast bf16. Core c receives
    np.roll(reps, -c*1024, axis=0) so every core runs the same static
    program on "its" first 1024 rows: self-similarity for local row li
    sits at column li, the positive partner at column li+4096.
  - On-chip per core: normalize rows (f32 stats, bf16 data), transpose via
    PE into repsT [D, N] (bf16), then the [1024, 8192] block of the
    similarity matrix as 128x1024 PSUM tiles (bf16 matmul, f32 accum).
    Self column is masked with a -1e30 eye tile; per tile the row-max runs
    on DVE and exp(4*sim-4) row-sums are fused into one ScalarE
    activation+accum.
  - Host: combine per-core stats (positives, hardest negatives, exp sums)
    in float64 into the scalar loss (the two "all-reduced" loss terms).
"""

import numpy as np
import ml_dtypes

import concourse.bacc as bacc
import concourse.bass as bass
import concourse.tile as tile
import concourse.mybir as mybir
from concourse.bass_utils import run_bass_kernel_spmd

B = 4096
D = 512
N = 2 * B            # 8192 rows total
NCORES = 8
NLOC = N // NCORES   # 1024 rows per core
RT = N // 128        # 64 row tiles
MT = NLOC // 128     # 8 local row tiles
NNW = 1024           # column super-tile width (2 PSUM banks)
NN = N // NNW        # 8 column super-tiles
KT = D // 128        # 4 contraction chunks

F32 = mybir.dt.float32
I32 = mybir.dt.int32
BF16 = mybir.dt.bfloat16

_CACHE = {}


def _build_program():
    if "nc" in _CACHE:
        return _CACHE["nc"]
    nc = bacc.Bacc(
        "TRN2",
        target_bir_lowering=False,
        debug=False,
        num_devices=NCORES,
    )

    z = nc.dram_tensor("z", [N, D], BF16, kind="ExternalInput").ap()
    ident = nc.dram_tensor("ident", [128, 128], BF16, kind="ExternalInput").ap()
    negeye = nc.dram_tensor("negeye", [128, 128], F32, kind="ExternalInput").ap()

    maxc_d = nc.dram_tensor("maxc", [MT, 128, NN], F32, kind="ExternalOutput").ap()
    esum_d = nc.dram_tensor("esum", [MT, 128, NN], F32, kind="ExternalOutput").ap()
    posd_d = nc.dram_tensor("posd", [128, MT], F32, kind="ExternalOutput").ap()
    invn_d = nc.dram_tensor("invn", [128, RT], F32, kind="ExternalOutput").ap()

    AX = mybir.AxisListType
    ALU = mybir.AluOpType
    AF = mybir.ActivationFunctionType

    with tile.TileContext(nc) as tc:
        with (
            tc.tile_pool(name="persist", bufs=1) as persist,
            tc.tile_pool(name="nrows", bufs=3) as nrows,
            tc.tile_pool(name="trash", bufs=2) as trashp,
            tc.tile_pool(name="etrash", bufs=2) as etrashp,
            tc.tile_pool(name="pstr", bufs=2, space="PSUM") as pstrp,
            tc.tile_pool(name="mm", bufs=3, space="PSUM") as mmp,
        ):
            zfull = persist.tile([128, RT, 512], BF16, tag="zfull")
            repsT = persist.tile([128, KT, N], BF16, tag="repsT")
            identS = persist.tile([128, 128], BF16, tag="identS")
            negeyeS = persist.tile([128, 128], F32, tag="negeyeS")
            ssqall = persist.tile([128, RT], F32, tag="ssqall")
            nrmall = persist.tile([128, RT], F32, tag="nrmall")
            invall = persist.tile([128, RT], F32, tag="invall")
            posdt = persist.tile([128, MT], F32, tag="posdt")
            negfour = persist.tile([128, 1], F32, tag="negfour")

            nc.vector.memset(negfour, -4.0)
            nc.sync.dma_start(out=identS, in_=ident)
            nc.sync.dma_start(out=negeyeS, in_=negeye)

            # per-m stat accumulators: column g holds stats of the g-th
            # 1024-wide column super-tile. maxm holds max of exp(4*sim-4)
            # (bf16, from the exp tile); host takes log to recover sim max.
            maxm = [
                persist.tile([128, NN], F32, tag=f"maxm{m}", name=f"maxm{m}")
                for m in range(MT)
            ]
            esm = [
                persist.tile([128, NN], F32, tag=f"esm{m}", name=f"esm{m}")
                for m in range(MT)
            ]

            def prep_dma(g):
                for r in range(g * 8, g * 8 + 8):
                    nc.sync.dma_start(
                        out=zfull[:, r, :], in_=z[r * 128 : (r + 1) * 128, :]
                    )

            def prep_load(g):
                """squared norms + inv + normalized rows for group g."""
                for r in range(g * 8, g * 8 + 8):
                    tr = trashp.tile([128, NNW], BF16, tag="trash")
                    nc.scalar.activation(
                        out=tr[:, :512],
                        in_=zfull[:, r, :],
                        func=AF.Square,
                        accum_out=ssqall[:, r : r + 1],
                    )
                gs = slice(g * 8, g * 8 + 8)
                nc.scalar.sqrt(nrmall[:, gs], ssqall[:, gs])
                nc.vector.reciprocal(invall[:, gs], nrmall[:, gs])
                for r in range(g * 8, g * 8 + 8):
                    nrow = nrows.tile([128, 512], BF16, tag="nrow")
                    nc.vector.tensor_scalar_mul(
                        nrow, zfull[:, r, :], invall[:, r : r + 1]
                    )
                    yield r, nrow

            def prep_transpose(r, nrow):
                pstr = pstrp.tile([128, KT, 128], BF16, tag="pstr")
                for k in range(KT):
                    nc.tensor.transpose(
                        pstr[:, k, :], nrow[:, k * 128 : (k + 1) * 128], identS
                    )
                nc.vector.tensor_copy(
                    out=repsT[:, :, r * 128 : (r + 1) * 128], in_=pstr
                )

            def main_group(g):
                """column super-tile g of the sim block, all m."""
                for m in range(MT):
                    ps = mmp.tile([128, NNW], F32, tag="ps")
                    for h in (0, 1):
                        for k in range(KT):
                            nc.tensor.matmul(
                                ps[:, h * 512 : (h + 1) * 512],
                                lhsT=repsT[:, k, m * 128 : (m + 1) * 128],
                                rhs=repsT[
                                    :, k, g * NNW + h * 512 : g * NNW + (h + 1) * 512
                                ],
                                start=(k == 0),
                                stop=(k == KT - 1),
                            )
                    if g == 0:
                        # mask self-similarity: sim[p, m*128+p] -= 1e30
                        nc.vector.tensor_add(
                            ps[:, m * 128 : (m + 1) * 128],
                            ps[:, m * 128 : (m + 1) * 128],
                            negeyeS,
                        )
                    et = etrashp.tile([128, NNW], BF16, tag="etrash")
                    nc.scalar.activation(
                        out=et,
                        in_=ps,
                        func=AF.Exp,
                        bias=negfour,
                        scale=4.0,
                        accum_out=esm[m][:, g : g + 1],
                    )
                    nc.vector.reduce_max(maxm[m][:, g : g + 1], et, axis=AX.X)
                # positives (raw bf16 dots of rolled rows q, q+32), two per
                # group once their partner group is resident
                if g >= 4:
                    for q in (2 * (g - 4), 2 * (g - 4) + 1):
                        prod = nrows.tile([128, 512], F32, tag="prod")
                        nc.vector.tensor_mul(
                            prod, zfull[:, q, :], zfull[:, q + 32, :]
                        )
                        nc.vector.reduce_sum(posdt[:, q : q + 1], prod, axis=AX.X)

            # ---- software-pipelined schedule: prep one group ahead,
            # DMA two groups ahead ----
            prep_dma(0)
            prep_dma(1)
            for r, nrow in prep_load(0):
                prep_transpose(r, nrow)
            for g in range(NN):
                if g + 2 < NN:
                    prep_dma(g + 2)
                pending = list(prep_load(g + 1)) if g + 1 < NN else []
                main_group(g)
                for r, nrow in pending:
                    prep_transpose(r, nrow)

            for m in range(MT):
                nc.sync.dma_start(out=maxc_d[m], in_=maxm[m])
                nc.sync.dma_start(out=esum_d[m], in_=esm[m])
            nc.sync.dma_start(out=posd_d, in_=posdt)
            nc.sync.dma_start(out=invn_d, in_=invall)

    nc.compile()
    _CACHE["nc"] = nc
    return nc


def _host_inputs(z_i, z_j):
    reps = np.concatenate(
        [np.asarray(z_i, np.float32), np.asarray(z_j, np.float32)], axis=0
    )
    zb = reps.astype(ml_dtypes.bfloat16)
    ident = np.eye(128, dtype=np.float32).astype(ml_dtypes.bfloat16)
    negeye = (np.eye(128, dtype=np.float32) * -1.0e30).astype(np.float32)
    in_maps = []
    for c in range(NCORES):
        zc = np.ascontiguousarray(np.roll(zb, -c * NLOC, axis=0))
        in_maps.append({"z": zc, "ident": ident, "negeye": negeye})
    return in_maps


def _combine(results):
    pos = np.zeros(N, np.float64)
    hn = np.zeros(N, np.float64)
    S = 0.0
    for c, o in enumerate(results):
        maxc = np.asarray(o["maxc"], np.float64)   # [MT, 128, NN]
        esum = np.asarray(o["esum"], np.float64)   # [MT, 128, NN]
        posd = np.asarray(o["posd"], np.float64)   # [128, MT]
        invn = np.asarray(o["invn"], np.float64)   # [128, RT]
        # maxc holds max over exp(4*sim-4) per column super-tile (bf16
        # rounded); invert the exp to recover the sim max.
        hn_loc = (np.log(maxc.max(axis=2).reshape(NLOC)) + 4.0) / 4.0
        S += esum.sum()                            # self terms exp'd to 0
        invrow = invn.T.reshape(N)                 # rolled row index
        posl = posd.T.reshape(NLOC) * invrow[:NLOC] * invrow[B : B + NLOC]
        gl = (np.arange(NLOC) + c * NLOC) % N
        pos[gl] = posl
        hn[gl] = hn_loc
    ce = np.mean(np.logaddexp(0.0, 40.0 * hn - 20.0 * pos))
    npairs = N * (N - 1) // 2
    uniformity = np.log(S / 2.0 / npairs)
    return np.array(ce + 0.2 * uniformity, dtype=np.float32)


def run(z_i, z_j, **spmd_kwargs):
    nc = _build_program()
    in_maps = _host_inputs(z_i, z_j)
    res = run_bass_kernel_spmd(nc, in_maps, core_ids=list(range(NCORES)), **spmd_kwargs)
    return _combine(res.results), res


def kernel(z_i, z_j):
    loss, _ = run(z_i, z_j)
    return loss



# revision 4
# speedup vs baseline: 1.0410x; 1.0410x over previous
"""NT-Xent loss kernel for 8 Trainium2 NeuronCores (Bass/Tile).

Strategy (data-parallel rows, SPMD, fp8 matmul):
  - Host: concat + L2-normalize z_i,z_j in f32 -> reps [8192, 512];
    positives computed exactly on host (f64). repsT = reps.T quantized to
    fp8 e4m3, laid out [128, 4, 8192] (partition, k-subtile, column).
    Core c receives np.roll(repsT, -c*1024, axis=2) so every core runs the
    same static program on "its" first 1024 rows: the self-similarity for
    local row li sits at column li.
  - On-chip per core: the [1024, 8192] block of the similarity matrix as
    [128, 2048] PSUM tiles via fp8 DoubleRow matmuls (2x bf16 throughput,
    f32 accum). Self column masked with a -1e30 eye tile (DVE); ScalarE
    computes exp(4*sim-4) -> bf16; DVE takes per-row max (hard negatives)
    and per-row sum (uniformity partial).
  - Host: combine per-core stats in float64 into the scalar loss.
"""

import numpy as np
import ml_dtypes

import concourse.bacc as bacc
import concourse.bass as bass
import concourse.tile as tile
import concourse.mybir as mybir
from concourse.bass_utils import run_bass_kernel_spmd

B = 4096
D = 512
N = 2 * B            # 8192 rows total
NCORES = 8
NLOC = N // NCORES   # 1024 rows per core
MT = NLOC // 128     # 8 local row tiles
GW = 2048            # column group width (4 PSUM banks)
NG = N // GW         # 4 column groups
KT = D // 128        # 4 k-subtiles
NST = MT * NG        # 32 stat columns per core

F32 = mybir.dt.float32
BF16 = mybir.dt.bfloat16
FP8 = mybir.dt.float8e4

_CACHE = {}


def _build_program():
    if "nc" in _CACHE:
        return _CACHE["nc"]
    nc = bacc.Bacc(
        "TRN2",
        target_bir_lowering=False,
        debug=False,
        num_devices=NCORES,
    )

    repsT_d = nc.dram_tensor("repsT", [128, KT, N], FP8, kind="ExternalInput").ap()
    negeye_d = nc.dram_tensor("negeye", [128, 128], F32, kind="ExternalInput").ap()

    maxm_d = nc.dram_tensor("maxm", [128, NST], F32, kind="ExternalOutput").ap()
    esum_d = nc.dram_tensor("esum", [128, NST], F32, kind="ExternalOutput").ap()

    AX = mybir.AxisListType
    AF = mybir.ActivationFunctionType
    PM = mybir.MatmulPerfMode.DoubleRow

    with tile.TileContext(nc) as tc:
        with (
            tc.tile_pool(name="persist", bufs=1) as persist,
            tc.tile_pool(name="et", bufs=3) as etp,
            tc.tile_pool(name="mm", bufs=2, space="PSUM") as mmp,
        ):
            repsT = persist.tile([128, KT, N], FP8, tag="repsT")
            negeyeS = persist.tile([128, 128], F32, tag="negeyeS")
            maxmS = persist.tile([128, NST], F32, tag="maxmS")
            esumS = persist.tile([128, NST], F32, tag="esumS")
            negfour = persist.tile([128, 1], F32, tag="negfour")

            nc.vector.memset(negfour, -4.0)
            nc.sync.dma_start(out=negeyeS, in_=negeye_d)
            # input DMA in 512-column chunks spread across queues
            for j in range(16):
                nc.sync.dma_start(
                    out=repsT[:, :, j * 512 : (j + 1) * 512],
                    in_=repsT_d[:, :, j * 512 : (j + 1) * 512],
                )

            for g in range(NG):
                for m in range(MT):
                    ps = mmp.tile([128, GW], F32, tag="ps")
                    lhsT0 = repsT[:, 0:2, m * 128 : (m + 1) * 128]
                    lhsT1 = repsT[:, 2:4, m * 128 : (m + 1) * 128]
                    for c4 in range(GW // 512):
                        cols = g * GW + c4 * 512
                        nc.tensor.matmul(
                            ps[:, c4 * 512 : (c4 + 1) * 512],
                            lhsT=lhsT0,
                            rhs=repsT[:, 0:2, cols : cols + 512],
                            start=True,
                            stop=False,
                            perf_mode=PM,
                        )
                    for c4 in range(GW // 512):
                        cols = g * GW + c4 * 512
                        nc.tensor.matmul(
                            ps[:, c4 * 512 : (c4 + 1) * 512],
                            lhsT=lhsT1,
                            rhs=repsT[:, 2:4, cols : cols + 512],
                            start=False,
                            stop=True,
                            perf_mode=PM,
                        )
                    if g == 0:
                        # mask self-similarity: sim[p, m*128+p] -= 1e30
                        nc.vector.tensor_add(
                            ps[:, m * 128 : (m + 1) * 128],
                            ps[:, m * 128 : (m + 1) * 128],
                            negeyeS,
                        )
                    et = etp.tile([128, GW], BF16, tag="et")
                    nc.scalar.activation(
                        out=et, in_=ps, func=AF.Exp, bias=negfour, scale=4.0
                    )
                    col = g * MT + m
                    nc.vector.reduce_max(
                        maxmS[:, col : col + 1], et, axis=AX.X
                    )
                    nc.vector.reduce_sum(
                        esumS[:, col : col + 1], et, axis=AX.X
                    )

            nc.sync.dma_start(out=maxm_d, in_=maxmS)
            nc.sync.dma_start(out=esum_d, in_=esumS)

    nc.compile()
    _CACHE["nc"] = nc
    return nc


def _host_prep(z_i, z_j):
    reps = np.concatenate(
        [np.asarray(z_i, np.float32), np.asarray(z_j, np.float32)], axis=0
    )
    reps = reps / np.maximum(
        np.linalg.norm(reps, axis=1, keepdims=True), 1e-12
    )
    pos = np.einsum(
        "ij,ij->i",
        reps.astype(np.float64),
        np.roll(reps, -B, axis=0).astype(np.float64),
    )  # pos[i] = reps[i] . reps[(i+B) % N]
    # [512, 8192] -> [4, 128, 8192] -> [128, 4, 8192]
    repsT = np.ascontiguousarray(
        reps.T.reshape(KT, 128, N).transpose(1, 0, 2)
    ).astype(ml_dtypes.float8_e4m3)
    negeye = (np.eye(128, dtype=np.float32) * -1.0e30).astype(np.float32)
    in_maps = []
    for c in range(NCORES):
        rc = np.ascontiguousarray(np.roll(repsT, -c * NLOC, axis=2))
        in_maps.append({"repsT": rc, "negeye": negeye})
    return in_maps, pos


def _combine(results, pos):
    hn = np.zeros(N, np.float64)
    S = 0.0
    for c, o in enumerate(results):
        maxm = np.asarray(o["maxm"], np.float64)  # [128, NG*MT]
        esum = np.asarray(o["esum"], np.float64)
        S += esum.sum()  # self terms exp'd to 0
        # maxm[p, g*MT+m] is max of exp(4*sim-4) for local row m*128+p,
        # cols [g*2048, (g+1)*2048); invert the exp to recover sim max.
        mx = maxm.reshape(128, NG, MT).max(axis=1)  # [128, MT]
        hn_loc = (np.log(mx.T.reshape(NLOC)) + 4.0) / 4.0
        gl = (np.arange(NLOC) + c * NLOC) % N
        hn[gl] = hn_loc
    ce = np.mean(np.logaddexp(0.0, 40.0 * hn - 20.0 * pos))
    npairs = N * (N - 1) // 2
    uniformity = np.log(S / 2.0 / npairs)
    return np.array(ce + 0.2 * uniformity, dtype=np.float32)


def run(z_i, z_j, **spmd_kwargs):
    nc = _build_program()
    in_maps, pos = _host_prep(z_i, z_j)
    res = run_bass_kernel_spmd(
        nc, in_maps, core_ids=list(range(NCORES)), **spmd_kwargs
    )
    return _combine(res.results, pos), res


def kernel(z_i, z_j):
    loss, _ = run(z_i, z_j)
    return loss


# revision 6
# speedup vs baseline: 2.0210x; 1.9415x over previous
"""NT-Xent loss kernel for 8 Trainium2 NeuronCores (Bass/Tile).

Strategy (data-parallel rows, SPMD, fp8 matmul):
  - Host: concat + L2-normalize z_i,z_j in f32 -> reps [8192, 512];
    positives computed exactly on host (f64). repsT = reps.T quantized to
    fp8 e4m3, laid out [128, 4, 8192] (partition, k-subtile, column).
    Core c receives np.roll(repsT, -c*1024, axis=2) so every core runs the
    same static program on "its" first 1024 rows: the self-similarity for
    local row li sits at column li.
  - On-chip per core: the [1024, 8192] block of the similarity matrix as
    [128, 2048] PSUM tiles via fp8 DoubleRow matmuls (2x bf16 throughput,
    f32 accum). Self column masked with a -1e30 eye tile (DVE); ScalarE
    computes exp(4*sim-4) -> bf16; DVE takes per-row max (hard negatives)
    and per-row sum (uniformity partial).
  - Host: combine per-core stats in float64 into the scalar loss.
"""

import numpy as np
import ml_dtypes

import concourse.bacc as bacc
import concourse.bass as bass
import concourse.tile as tile
import concourse.mybir as mybir
from concourse.bass_utils import run_bass_kernel_spmd

B = 4096
D = 512
N = 2 * B            # 8192 rows total
NCORES = 8
NLOC = N // NCORES   # 1024 rows per core
MT = NLOC // 128     # 8 local row tiles
GW = 2048            # column group width (4 PSUM banks)
NG = N // GW         # 4 column groups
KT = D // 128        # 4 k-subtiles
NST = MT * NG        # 32 stat columns per core

F32 = mybir.dt.float32
BF16 = mybir.dt.bfloat16
FP8 = mybir.dt.float8e4

_CACHE = {}


def _build_program():
    if "nc" in _CACHE:
        return _CACHE["nc"]
    nc = bacc.Bacc(
        "TRN2",
        target_bir_lowering=False,
        debug=False,
        num_devices=NCORES,
    )

    repsT_d = nc.dram_tensor("repsT", [128, KT, N], FP8, kind="ExternalInput").ap()
    negeye_d = nc.dram_tensor("negeye", [128, 128], F32, kind="ExternalInput").ap()

    maxm_d = nc.dram_tensor("maxm", [128, MT], F32, kind="ExternalOutput").ap()
    esum_d = nc.dram_tensor("esum", [128, NST], F32, kind="ExternalOutput").ap()

    AX = mybir.AxisListType
    AF = mybir.ActivationFunctionType
    ALU = mybir.AluOpType
    PM = mybir.MatmulPerfMode.DoubleRow

    with tile.TileContext(nc) as tc:
        with (
            tc.tile_pool(name="persist", bufs=1) as persist,
            tc.tile_pool(name="et", bufs=3) as etp,
            tc.tile_pool(name="fold", bufs=2) as foldp,
            tc.tile_pool(name="mm", bufs=2, space="PSUM") as mmp,
        ):
            repsT = persist.tile([128, KT, N], FP8, tag="repsT")
            negeyeS = persist.tile([128, 128], F32, tag="negeyeS")
            maxmS = persist.tile([128, MT], F32, tag="maxmS")
            esumS = persist.tile([128, NST], F32, tag="esumS")
            negfour = persist.tile([128, 1], F32, tag="negfour")
            accs = [
                persist.tile([128, GW], BF16, tag=f"acc{m}", name=f"acc{m}")
                for m in range(MT)
            ]

            nc.vector.memset(negfour, -4.0)
            nc.sync.dma_start(out=negeyeS, in_=negeye_d)
            # input DMA in 512-column chunks spread across queues
            for j in range(16):
                nc.sync.dma_start(
                    out=repsT[:, :, j * 512 : (j + 1) * 512],
                    in_=repsT_d[:, :, j * 512 : (j + 1) * 512],
                )

            for g in range(NG):
                for m in range(MT):
                    ps = mmp.tile([128, GW], F32, tag="ps")
                    lhsT0 = repsT[:, 0:2, m * 128 : (m + 1) * 128]
                    lhsT1 = repsT[:, 2:4, m * 128 : (m + 1) * 128]
                    for c4 in range(GW // 512):
                        cols = g * GW + c4 * 512
                        nc.tensor.matmul(
                            ps[:, c4 * 512 : (c4 + 1) * 512],
                            lhsT=lhsT0,
                            rhs=repsT[:, 0:2, cols : cols + 512],
                            start=True,
                            stop=False,
                            perf_mode=PM,
                        )
                    for c4 in range(GW // 512):
                        cols = g * GW + c4 * 512
                        nc.tensor.matmul(
                            ps[:, c4 * 512 : (c4 + 1) * 512],
                            lhsT=lhsT1,
                            rhs=repsT[:, 2:4, cols : cols + 512],
                            start=False,
                            stop=True,
                            perf_mode=PM,
                        )
                    if g == 0:
                        # mask self-similarity: sim[p, m*128+p] -= 1e30
                        nc.vector.tensor_add(
                            ps[:, m * 128 : (m + 1) * 128],
                            ps[:, m * 128 : (m + 1) * 128],
                            negeyeS,
                        )
                    et = etp.tile([128, GW], BF16, tag="et")
                    col = g * MT + m
                    nc.scalar.activation(
                        out=et,
                        in_=ps,
                        func=AF.Exp,
                        bias=negfour,
                        scale=4.0,
                        accum_out=esumS[:, col : col + 1],
                    )
                    # running per-row max of exp tiles (tensor_tensor runs
                    # in the 2x DVE mode; tensor_reduce would not)
                    if g == 0:
                        nc.vector.tensor_copy(out=accs[m], in_=et)
                    else:
                        nc.vector.tensor_tensor(
                            out=accs[m], in0=accs[m], in1=et, op=ALU.max
                        )

            # fold accs [128, 2048] -> [128, 256] -> reduce -> maxmS col m
            for m in range(MT):
                f1 = foldp.tile([128, 1024], BF16, tag="f1")
                nc.vector.tensor_tensor(
                    out=f1, in0=accs[m][:, :1024], in1=accs[m][:, 1024:], op=ALU.max
                )
                nc.vector.tensor_tensor(
                    out=f1[:, :512], in0=f1[:, :512], in1=f1[:, 512:], op=ALU.max
                )
                nc.vector.tensor_tensor(
                    out=f1[:, :256], in0=f1[:, :256], in1=f1[:, 256:512], op=ALU.max
                )
                nc.vector.reduce_max(
                    maxmS[:, m : m + 1], f1[:, :256], axis=AX.X
                )

            nc.sync.dma_start(out=maxm_d, in_=maxmS)
            nc.sync.dma_start(out=esum_d, in_=esumS)

    nc.compile()
    _CACHE["nc"] = nc
    return nc


def _host_prep(z_i, z_j):
    reps = np.concatenate(
        [np.asarray(z_i, np.float32), np.asarray(z_j, np.float32)], axis=0
    )
    reps = reps / np.maximum(
        np.linalg.norm(reps, axis=1, keepdims=True), 1e-12
    )
    pos = np.einsum(
        "ij,ij->i",
        reps.astype(np.float64),
        np.roll(reps, -B, axis=0).astype(np.float64),
    )  # pos[i] = reps[i] . reps[(i+B) % N]
    # [512, 8192] -> [4, 128, 8192] -> [128, 4, 8192]
    repsT = np.ascontiguousarray(
        reps.T.reshape(KT, 128, N).transpose(1, 0, 2)
    ).astype(ml_dtypes.float8_e4m3)
    negeye = (np.eye(128, dtype=np.float32) * -1.0e30).astype(np.float32)
    in_maps = []
    for c in range(NCORES):
        rc = np.ascontiguousarray(np.roll(repsT, -c * NLOC, axis=2))
        in_maps.append({"repsT": rc, "negeye": negeye})
    return in_maps, pos


def _combine(results, pos):
    hn = np.zeros(N, np.float64)
    S = 0.0
    for c, o in enumerate(results):
        maxm = np.asarray(o["maxm"], np.float64)  # [128, MT]
        esum = np.asarray(o["esum"], np.float64)
        S += esum.sum()  # self terms exp'd to 0
        # maxm[p, m] is max of exp(4*sim-4) for local row m*128+p;
        # invert the exp to recover the sim max.
        hn_loc = (np.log(maxm.T.reshape(NLOC)) + 4.0) / 4.0
        gl = (np.arange(NLOC) + c * NLOC) % N
        hn[gl] = hn_loc
    ce = np.mean(np.logaddexp(0.0, 40.0 * hn - 20.0 * pos))
    npairs = N * (N - 1) // 2
    uniformity = np.log(S / 2.0 / npairs)
    return np.array(ce + 0.2 * uniformity, dtype=np.float32)


def run(z_i, z_j, **spmd_kwargs):
    nc = _build_program()
    in_maps, pos = _host_prep(z_i, z_j)
    res = run_bass_kernel_spmd(
        nc, in_maps, core_ids=list(range(NCORES)), **spmd_kwargs
    )
    return _combine(res.results, pos), res


def kernel(z_i, z_j):
    loss, _ = run(z_i, z_j)
    return loss
